# revision 23
# baseline (speedup 1.0000x reference)
"""
Distributed GQA attention block for Trainium2 (8 NeuronCores).

Problem: out = AttentionBlock(x; wq, wk, wv, wo)
  B=2, S=2048, DIM=4096, n_heads=32, n_kv_heads=8, head_dim=128,
  rope theta=5e5, causal, softmax, f32 I/O.

Sharding strategy (tensor-parallel over heads, per-chunk ReduceScatter):
  - Each core c owns 4 query heads (4c..4c+3) and 1 kv head (c).
  - Per core: q/k/v projections for its heads (column shards of wq/wk/wv),
    RoPE, causal attention for its 4 heads over the full sequence.
  - Output projection is ROW-sharded: core c computes the partial product
    attn_own^T @ wo[512c:512c+512, :] -> partial [T, DIM] (bf16), and a
    ReduceScatter(add) per 512-token chunk sums the partials across cores,
    leaving each core with a 64-token slice of the final output per chunk.
    Host-side unshard is a pure concatenation (no host compute).

Pipeline (in-order engines; emission order == execution order per engine):
  Phase A: projections+rope for batch 0 (q,k,v) and batch 1 (k,v only).
  Phase B: batch-1 Q projection interleaved with batch-0 attention
           (attention exp runs on ACT while PE does projection matmuls).
  Phase C: batch-0 output projection interleaved with batch-1 attention;
           per-chunk ReduceScatter of batch-0 partials.
  Phase D: batch-1 output projection + per-chunk ReduceScatter.
The per-chunk ReduceScatters (28us each) hide under compute; only the
last chunk's collective is exposed (~30us tail), vs. 2x265us AllGathers
that stalled the PE for ~340us in the AllGather design.

Compute dtype: bf16 operands with f32 PSUM accumulation. Softmax skips
the max-subtraction (scores are < ~15 at this problem's scale), the
denominator comes free from an appended ones-column in the PV matmul,
and normalization is applied to the [tok, 128] attention output instead
of the [tok, 2048] probabilities.

RoPE layout trick: wq/wk columns are host-permuted so each head's even
dims come first and odd dims second. The rotation's pair swap then
becomes two 64-partition block copies (SBUF->SBUF DMA) instead of a
cross-partition interleave.
"""

import math
from contextlib import ExitStack
from types import SimpleNamespace

import numpy as np
import ml_dtypes

P = 128
BF16 = ml_dtypes.bfloat16


_CACHE = {}
_TRACE = False


def make_cfg(B=2, S=2048, DIM=4096, H=32, KVH=8, HD=128, THETA=500000.0,
             NCORES=8):
    c = SimpleNamespace(B=B, S=S, DIM=DIM, H=H, KVH=KVH, HD=HD, THETA=THETA,
                        NCORES=NCORES)
    c.T = B * S
    c.HPC = H // NCORES          # query heads per core
    c.QF = c.HPC * HD            # query features per core
    c.SCALE = 1.0 / math.sqrt(HD)
    c.TCH = 512                  # token chunk
    c.NKT = DIM // P             # contraction tiles
    c.NTT = c.T // P             # token tiles
    c.NCH = c.T // c.TCH         # token chunks
    c.SQT = S // P               # q/k tiles per sequence
    c.VW = HD + 1                # v + ones column
    c.RST = c.TCH // NCORES      # tokens per core per ReduceScatter chunk
    # ReduceScatter segments (t0, ntok): even 512s keep the serialized
    # collective chain ahead of the partial-sum arrivals
    c.SEGS = [(0, 512), (512, 512), (1024, 512), (1536, 512),
              (2048, 512), (2560, 768), (3328, 768)]
    assert sum(n for _, n in c.SEGS) == c.T
    assert all(n % NCORES == 0 and n % P == 0 for _, n in c.SEGS)
    assert S % c.TCH == 0 and c.T % c.TCH == 0 and DIM % P == 0
    # each core's HPC query heads share the core's single kv head
    assert KVH == NCORES and c.HPC == H // KVH
    assert c.HPC * HD == c.TCH  # feature tiles per core == heads
    return c


def _build_graph(c, phases=4):
    """Build + compile the SPMD Bass graph (same program on every core)."""
    import concourse.mybir as mybir
    import concourse.tile as tile
    from concourse import bacc

    fp32 = mybir.dt.float32
    bf16 = mybir.dt.bfloat16

    nc = bacc.Bacc(
        "TRN2",
        target_bir_lowering=False,
        debug=False,
        enable_asserts=True,
        num_devices=c.NCORES,
    )

    # ---- kernel I/O ----
    xT = nc.dram_tensor("xT", [c.DIM, c.T], bf16, kind="ExternalInput").ap()
    wq = nc.dram_tensor("wq", [c.DIM, c.QF], bf16, kind="ExternalInput").ap()
    wkv = nc.dram_tensor("wkv", [c.DIM, 2 * c.HD], bf16,
                         kind="ExternalInput").ap()
    # row shard of the output projection: [own 512 features, DIM]
    wo = nc.dram_tensor("wo", [c.QF, c.DIM], bf16, kind="ExternalInput").ap()
    cosi = nc.dram_tensor("cosi", [P, c.S], fp32, kind="ExternalInput").ap()
    sini = nc.dram_tensor("sini", [P, c.S], fp32, kind="ExternalInput").ap()
    tril = nc.dram_tensor("tril", [P, P], bf16, kind="ExternalInput").ap()
    ident = nc.dram_tensor("ident", [P, P], fp32, kind="ExternalInput").ap()
    # per-chunk ReduceScatter shards, concatenated: row ch*RST+r holds
    # token ch*TCH + rank*RST + r of the final output
    out = nc.dram_tensor("out", [c.NCH * c.RST, c.DIM], bf16,
                         kind="ExternalOutput").ap()

    Exp = mybir.ActivationFunctionType.Exp
    Copy = mybir.ActivationFunctionType.Copy
    TPP = c.TCH // P          # token sub-tiles per chunk
    CPB = c.NCH // c.B        # token chunks per batch
    KG = 4                    # contraction tiles fetched per DMA
    NQT = c.HPC + 1           # max rope targets per chunk (4 q + 1 k)
    OCB = 8                   # 512-col blocks of DIM in output projection

    with tile.TileContext(nc) as tc:
        # ------- static SBUF tensors -------
        qT_b, kT_b, v_b, free_stat = [], [], [], []
        for b in range(c.B):
            t_, f_ = tc.tile([P, c.HPC, c.S], bf16, name=f"qT_sb{b}")
            qT_b.append(t_); free_stat.append(f_)
            t_, f_ = tc.tile([P, c.S], bf16, name=f"kT_sb{b}")
            kT_b.append(t_); free_stat.append(f_)
            t_, f_ = tc.tile([P, c.SQT, c.VW], bf16, name=f"v_sb{b}")
            v_b.append(t_); free_stat.append(f_)
        cos_sb, free_cos = tc.tile([P, c.S], fp32, name="cos_sb")
        sin_sb, free_sin = tc.tile([P, c.S], fp32, name="sin_sb")
        free_stat += [free_cos, free_sin]
        tril_sb, free_tril = tc.tile([P, P], bf16, name="tril_sb")
        id_sb, free_id = tc.tile([P, P], fp32, name="id_sb")
        id16_sb, free_id16 = tc.tile([P, P], bf16, name="id16_sb")
        free_stat += [free_tril, free_id, free_id16]

        nc.scalar.dma_start(tril_sb[:], tril[:])
        nc.scalar.dma_start(id_sb[:], ident[:])
        nc.vector.tensor_copy(id16_sb[:], id_sb[:])
        for b in range(c.B):
            nc.vector.memset(v_b[b][:, :, c.HD:c.VW], 1.0)  # denominator ones

        # dummy exp at t=0: pulls the ~2.7us exp_and_others ACT-table load
        # off the attention critical path
        warm_sb, free_warm = tc.tile([1, 1], fp32, name="warm_sb")
        nc.scalar.activation(warm_sb[:], id_sb[0:1, 0:1], Exp)
        free_stat.append(free_warm)

        with tc.tile_pool(name="dram", bufs=1, space="DRAM") as dramp:
            attnT_b = [
                dramp.tile([c.QF, c.S], bf16, name=f"attnT{b}")
                for b in range(c.B)
            ]
            partial = dramp.tile([c.T, c.DIM], bf16, name="partial")
            rs_out = dramp.tile([c.NCH * c.RST, c.DIM], bf16,
                                name="rs_out")

            # ---- weight pools (wop outlives wpool -> allocate first) ----
            es_wo = ExitStack()
            wop = es_wo.enter_context(tc.tile_pool(name="wop", bufs=1))
            wo_t = [
                wop.tile([P, c.DIM], bf16, tag="wo", bufs=c.HPC,
                         name=f"wo_t{f}")
                for f in range(c.HPC)
            ]

            es_w = ExitStack()
            wpool = es_w.enter_context(tc.tile_pool(name="wpool", bufs=1))

            def load_bulk(part):
                # bulk loads (6 MB) go on the gpsimd queue in ~0.7us pieces
                # spread across the A1 chunks: the ~1us/trigger Q7 descgen
                # cost plus the spreading keeps them from starving the
                # critical x/weight stream on the serialized DMA engines
                PC = 1024
                if part == 0:
                    for pc in range(c.S // PC):
                        sl = slice(pc * PC, (pc + 1) * PC)
                        nc.gpsimd.dma_start(cos_sb[:, sl], cosi[:, sl])
                        nc.gpsimd.dma_start(sin_sb[:, sl], sini[:, sl])
                else:
                    pcs = [(pc, f) for pc in range(c.DIM // PC)
                           for f in range(c.HPC)]
                    for pc, f in pcs[(part - 1) * 6:part * 6]:
                        sl = slice(pc * PC, (pc + 1) * PC)
                        nc.gpsimd.dma_start(wo_t[f][:, sl],
                                            wo[f * P:(f + 1) * P, sl])

            wq_t = [None] * c.NKT
            KVG = 8                   # wkv contraction tiles per DMA
            wkv_t = [None] * (c.NKT // KVG)

            def load_wkv(g):
                wt = wpool.tile([P, KVG, 2 * c.HD], bf16, tag="wkv",
                                bufs=c.NKT // KVG, name=f"wkv_t{g}")
                nc.scalar.dma_start(
                    wt[:],
                    wkv[g * KVG * P:(g + 1) * KVG * P, :].rearrange(
                        "(o p) f -> p o f", p=P),
                )
                wkv_t[g] = wt

            def wk_at(kt):
                return wkv_t[kt // KVG][:, kt % KVG, 0:c.HD]

            def wv_at(kt):
                return wkv_t[kt // KVG][:, kt % KVG, c.HD:2 * c.HD]

            def load_wq(kt):
                wqt = wpool.tile([P, c.QF], bf16, tag="wq", bufs=c.NKT,
                                 name=f"wq_t{kt}")
                nc.scalar.dma_start(wqt[:], wq[kt * P:(kt + 1) * P, :])
                wq_t[kt] = wqt

            # ---- projection-phase pools (A + B) ----
            es_pj = ExitStack()
            xpool = es_pj.enter_context(tc.tile_pool(name="xpool", bufs=3))
            ropep = es_pj.enter_context(tc.tile_pool(name="rope", bufs=1))
            es_kv = ExitStack()
            pjkvps = es_kv.enter_context(
                tc.tile_pool(name="pjkv_ps", bufs=1, space="PSUM"))

            first_q_chunk = [True]
            first_kv_chunk = [True]

            def ph1_chunk(ch, qpool, do_q, do_k, do_v):
                """Generator: projections (+rope) for one 512-token chunk.
                Yields after each contraction group (KG tiles)."""
                t0 = ch * c.TCH
                bch = ch // CPB
                lt0 = t0 - bch * c.S
                q_ps = [
                    qpool.tile([P, c.TCH], fp32, tag=f"q{ft}", bufs=1,
                               name=f"q_ps{ft}")
                    for ft in range(c.HPC)
                ] if do_q else None
                k_ps = pjkvps.tile([P, c.TCH], fp32, tag="k", bufs=1,
                                   name="k_ps") if do_k else None
                v_ps = pjkvps.tile([P, c.TCH], fp32, tag="v", bufs=1,
                                   name="v_ps") if do_v else None

                for kg in range(c.NKT // KG):
                    if do_k and first_kv_chunk[0] and kg % 2 == 0:
                        load_wkv(kg // 2)
                    if do_q and first_q_chunk[0]:
                        for kt in range(kg * KG, (kg + 1) * KG):
                            load_wq(kt)
                    xt4 = xpool.tile([P, KG, c.TCH], bf16, tag="xt")
                    nc.sync.dma_start(
                        xt4[:],
                        xT[kg * KG * P:(kg + 1) * KG * P,
                           t0:t0 + c.TCH].rearrange("(o p) t -> p o t", p=P),
                    )
                    for ki in range(KG):
                        kt = kg * KG + ki
                        xt = xt4[:, ki, :]
                        st = kt == 0
                        sp = kt == c.NKT - 1
                        if do_q:
                            for ft in range(c.HPC):
                                nc.tensor.matmul(
                                    q_ps[ft][:],
                                    lhsT=wq_t[kt][:, ft * P:(ft + 1) * P],
                                    rhs=xt, start=st, stop=sp,
                                )
                        if do_k:
                            nc.tensor.matmul(
                                k_ps[:], lhsT=wk_at(kt), rhs=xt,
                                start=st, stop=sp,
                            )
                        if do_v:
                            nc.tensor.matmul(
                                v_ps[:], lhsT=wv_at(kt), rhs=xt,
                                start=st, stop=sp,
                            )
                    yield

                if do_q:
                    first_q_chunk[0] = False
                if do_k:
                    first_kv_chunk[0] = False

                if do_v:
                    # vT (feature-major) -> bf16 sbuf, PE-transpose to
                    # token-major v_b
                    vt_sb = ropep.tile([P, c.TCH], bf16, tag="vt",
                                       bufs=2, name="vt_sb")
                    nc.scalar.activation(vt_sb[:], v_ps[:], Copy)
                    vtp = pjkvps.tile([P, 2, P], bf16, tag="vtp", bufs=1,
                                      name="vtp")
                    for sub in range(TPP):
                        gt = lt0 // P + sub
                        nc.tensor.transpose(
                            vtp[:, sub % 2, :],
                            vt_sb[:, sub * P:(sub + 1) * P], id16_sb[:],
                        )
                        nc.vector.tensor_copy(v_b[bch][:, gt, 0:c.HD],
                                              vtp[:, sub % 2, :])

                # ---- RoPE on present q tiles (+ k) ----
                nrt = (c.HPC if do_q else 0) + (1 if do_k else 0)
                if nrt == 0:
                    return
                ct = cos_sb[:, lt0:lt0 + c.TCH]
                st_t = sin_sb[:, lt0:lt0 + c.TCH]

                qbig = ropep.tile([P, NQT, c.TCH], fp32, tag="qbig",
                                  name="qbig")
                if do_q:
                    for ft in range(c.HPC):
                        if ft % 2 == 0:
                            nc.scalar.activation(qbig[:, ft, :], q_ps[ft][:],
                                                 Copy)
                        else:
                            nc.vector.tensor_copy(qbig[:, ft, :], q_ps[ft][:])
                if do_k:
                    nc.scalar.activation(qbig[:, nrt - 1, :], k_ps[:], Copy)

                qsw = ropep.tile([P, NQT, c.TCH], fp32, tag="qsw", name="qsw")
                # pair swap == half-partition block swap (even|odd split)
                nc.scalar.dma_start(qsw[0:64, 0:nrt, :],
                                    qbig[64:128, 0:nrt, :])
                nc.scalar.dma_start(qsw[64:128, 0:nrt, :],
                                    qbig[0:64, 0:nrt, :])

                ctb = ct[:, None, :].to_broadcast((P, nrt, c.TCH))
                stb = st_t[:, None, :].to_broadcast((P, nrt, c.TCH))
                nc.vector.tensor_mul(qbig[:, 0:nrt, :], qbig[:, 0:nrt, :], ctb)
                nc.vector.tensor_mul(qsw[:, 0:nrt, :], qsw[:, 0:nrt, :], stb)
                rr = ropep.tile([P, NQT, c.TCH], bf16, tag="rr", bufs=1,
                                name="rr")
                nc.vector.tensor_add(rr[:, 0:nrt, :], qbig[:, 0:nrt, :],
                                     qsw[:, 0:nrt, :])
                if do_q:
                    for ft in range(c.HPC):
                        nc.vector.tensor_copy(
                            qT_b[bch][:, ft, lt0:lt0 + c.TCH], rr[:, ft, :])
                if do_k:
                    nc.vector.tensor_copy(
                        kT_b[bch][:, lt0:lt0 + c.TCH], rr[:, nrt - 1, :])

            def ph2_units(b):
                """Generator: attention for batch b, one (head, qtile) unit
                per yield. Score tiles are computed lazily (at the step
                where first needed) to bound live prob-tile count."""
                for h in range(c.HPC):
                    qh = qT_b[b][:, h, :]
                    pts = {}
                    at4 = [None]
                    for qi in range(c.SQT):
                        q0 = qi * P
                        w = min(c.TCH, c.S - q0)
                        # new score tiles on this anti-diagonal:
                        # (kj, m) with kj + TPP*m == qi
                        new = [(qi - TPP * m, m) for m in range(qi // TPP + 1)]
                        # interleave: first 2 scores, then PVs over old
                        # tiles, then remaining scores, then PVs over new
                        # tiles (keeps PE fed while exp catches up)
                        newset = {kj for kj, _ in new}
                        old_kj = [kj for kj in range(qi + 1)
                                  if kj not in newset]
                        new_kj = sorted(newset)

                        def emit_score(kj, m):
                            s_ps = spool.tile([P, c.TCH], fp32, tag="s",
                                              name="s_ps")
                            nc.tensor.matmul(
                                s_ps[:, :w],
                                lhsT=kT_b[b][:, kj * P:(kj + 1) * P],
                                rhs=qh[:, q0:q0 + w],
                                start=True, stop=True,
                            )
                            pt = ppool.tile([P, c.TCH], bf16, tag="pt",
                                            name="pt")
                            nc.scalar.activation(pt[:, :w], s_ps[:, :w], Exp,
                                                 scale=c.SCALE)
                            if m == 0:
                                nc.vector.tensor_mul(
                                    pt[:, 0:P], pt[:, 0:P], tril_sb[:]
                                )
                            pts[(kj, m)] = pt

                        def emit_pv(kj, start, stop):
                            m = (qi - kj) // TPP
                            off = ((qi - kj) % TPP) * P
                            nc.tensor.matmul(
                                o_ps[:, qi % 2, :],
                                lhsT=pts[(kj, m)][:, off:off + P],
                                rhs=v_b[b][:, kj, :],
                                start=start, stop=stop,
                            )

                        for kj, m in new[:2]:
                            emit_score(kj, m)
                        first = True
                        for kj in old_kj:
                            emit_pv(kj, first, False)
                            first = False
                        for kj, m in new[2:]:
                            emit_score(kj, m)
                        for i, kj in enumerate(new_kj):
                            emit_pv(kj, first, i == len(new_kj) - 1)
                            first = False

                        rec = apool.tile([P, 1], fp32, tag="rec", name="rec")
                        nc.vector.reciprocal(rec[:],
                                             o_ps[:, qi % 2, c.HD:c.VW])
                        ao = apool.tile([P, P], bf16, tag="ao", name="ao")
                        nc.vector.tensor_scalar_mul(
                            ao[:], o_ps[:, qi % 2, 0:c.HD], rec[:]
                        )
                        nc.tensor.transpose(tp_ps[:, qi % 2, :], ao[:],
                                            id16_sb[:])
                        # stage TPP consecutive qi so the attnT write is one
                        # 128KB DMA
                        g4 = qi // TPP
                        if qi % TPP == 0:
                            at4[0] = a4pool.tile([P, c.TCH], bf16, tag="a4",
                                                 name="at4")
                        nc.vector.tensor_copy(
                            at4[0][:, (qi % TPP) * P:(qi % TPP + 1) * P],
                            tp_ps[:, qi % 2, :],
                        )
                        if qi % TPP == TPP - 1:
                            nc.scalar.dma_start(
                                attnT_b[b][h * P:(h + 1) * P,
                                           g4 * c.TCH:(g4 + 1) * c.TCH],
                                at4[0][:],
                            )
                        yield

            # ---- output-projection pools (C + D) ----
            def make_ph4_pools(es):
                wops = es.enter_context(
                    tc.tile_pool(name="wo_ps", bufs=4, space="PSUM"))
                atp = es.enter_context(tc.tile_pool(name="atp", bufs=2))
                stp = es.enter_context(tc.tile_pool(name="stp", bufs=2))
                return wops, atp, stp

            a4_pre = {}

            def prefetch_a4(pools, ch):
                if ch >= c.NCH:
                    return
                wops, atp, stp = pools
                bch = ch // CPB
                lt0 = ch * c.TCH - bch * c.S
                a4 = atp.tile([P, c.HPC, c.TCH], bf16, tag="a4r",
                              name="a4_pref")
                nc.sync.dma_start(
                    a4[:],
                    attnT_b[bch][:, lt0:lt0 + c.TCH].rearrange(
                        "(h p) t -> p h t", p=P),
                )
                a4_pre[ch] = a4

            def ph4_chunk(pools, ch, copy_engines):
                """Generator: row-parallel wo partial product for one chunk.
                Yields after each 128-token sub-tile (4 per chunk)."""
                wops, atp, stp = pools
                t0 = ch * c.TCH
                a4 = a4_pre.pop(ch)
                for sub in range(TPP):
                    stage = stp.tile([P, c.DIM], bf16, tag="st", name="stage")
                    for oc in range(OCB):
                        ops = wops.tile([P, c.TCH], fp32, tag="ob",
                                        name="o4_ps")
                        for f in range(c.HPC):
                            nc.tensor.matmul(
                                ops[:],
                                lhsT=a4[:, f, sub * P:(sub + 1) * P],
                                rhs=wo_t[f][:, oc * c.TCH:(oc + 1) * c.TCH],
                                start=(f == 0), stop=(f == c.HPC - 1),
                            )
                        cp = copy_engines[oc % len(copy_engines)]
                        cp(stage[:, oc * c.TCH:(oc + 1) * c.TCH], ops[:])
                    # batch-1 attnT is still being written by interleaved
                    # attention units during phase C: the first batch-1
                    # prefetch must be EMITTED after the last unit that
                    # writes its region (emission order == dep visibility)
                    late = (ch + 1) == CPB
                    if (sub == 0 and not late) or (sub == TPP - 1 and late):
                        prefetch_a4(pools, ch + 1)
                    nc.scalar.dma_start(
                        partial[t0 + sub * P:t0 + (sub + 1) * P, :],
                        stage[:],
                    )
                    yield

            def cp_vec(dst, srcap):
                nc.vector.tensor_copy(dst, srcap)

            def cp_pool(dst, srcap):
                nc.gpsimd.tensor_copy(dst, srcap)

            def cp_act(dst, srcap):
                nc.scalar.activation(dst, srcap, Copy)

            def emit_rs(t0, ntok, row0):
                nrow = ntok // c.NCORES
                nc.gpsimd.collective_compute(
                    "ReduceScatter",
                    mybir.AluOpType.add,
                    replica_groups=[list(range(c.NCORES))],
                    ins=[partial[t0:t0 + ntok, :].opt()],
                    outs=[rs_out[row0:row0 + nrow, :].opt()],
                )
                nc.gpsimd.dma_start(
                    out[row0:row0 + nrow, :],
                    rs_out[row0:row0 + nrow, :],
                )

            # ================= schedule =================
            # Phase A1: batch-0 q/k/v projections
            es_qA = ExitStack()
            pjqA = es_qA.enter_context(
                tc.tile_pool(name="pjqA_ps", bufs=1, space="PSUM"))
            for ch in range(CPB):
                for kg, _ in enumerate(ph1_chunk(ch, pjqA, True, True, True)):
                    if kg == 1:
                        load_bulk(ch)
                    elif ch > 0 and kg == 5:
                        load_bulk(CPB + ch)
            es_qA.close()

            # ---- attention pools (A2 + B + C) ----
            es_at = ExitStack()
            spool = es_at.enter_context(
                tc.tile_pool(name="spool", bufs=2, space="PSUM", side="right"))
            ovps = es_at.enter_context(
                tc.tile_pool(name="ovps", bufs=1, space="PSUM", side="right"))
            ppool = es_at.enter_context(
                tc.tile_pool(name="ppool", bufs=18, side="right"))
            apool = es_at.enter_context(
                tc.tile_pool(name="apool", bufs=4, side="right"))
            a4pool = es_at.enter_context(
                tc.tile_pool(name="at4", bufs=2, side="right"))

            o_ps = ovps.tile([P, 2, c.VW], fp32, tag="o", bufs=1, name="o_ps")
            tp_ps = ovps.tile([P, 2, P], bf16, tag="tp", bufs=1, name="tp_ps")

            # Phase A2: batch-1 k/v (x-DMA-bound) interleaved with the first
            # batch-0 attention units
            g2 = ph2_units(0) if phases >= 2 else iter(())
            for ch in range(CPB, 2 * CPB):
                for _ in ph1_chunk(ch, None, False, True, True):
                    next(g2, None)
            es_kv.close()

            # Phase B: batch-1 q projection interleaved with the rest of
            # batch-0 attention
            es_qB = ExitStack()
            pjqB = es_qB.enter_context(
                tc.tile_pool(name="pjqB_ps", bufs=1, space="PSUM"))
            for ch in range(CPB, 2 * CPB):
                for _ in ph1_chunk(ch, pjqB, True, False, False):
                    next(g2, None)
            for _ in g2:
                pass
            es_qB.close()
            es_pj.close()
            es_w.close()

            # Phase C: batch-0 output projection (+ ReduceScatter per
            # segment) interleaved with batch-1 attention
            es_p4 = ExitStack()
            pools4 = make_ph4_pools(es_p4)
            if phases >= 3:
                seg_iter = iter(c.SEGS)
                seg = next(seg_iter)
                row0 = 0

                def after_sub(tend):
                    # emit the ReduceScatter of any segment ending at tend
                    nonlocal seg, row0
                    if seg is not None and seg[0] + seg[1] == tend:
                        emit_rs(seg[0], seg[1], row0)
                        row0 += seg[1] // c.NCORES
                        seg = next(seg_iter, None)

                g2 = ph2_units(1) if phases >= 2 else iter(())
                prefetch_a4(pools4, 0)
                for ch in range(CPB):
                    for sub, _ in enumerate(ph4_chunk(pools4, ch,
                                                      (cp_vec, cp_act))):
                        for _ in range(4):
                            next(g2, None)
                        after_sub(ch * c.TCH + (sub + 1) * P)
                for _ in g2:
                    pass

                # Phase D: batch-1 output projection + ReduceScatter
                for ch in range(CPB, 2 * CPB):
                    for sub, _ in enumerate(ph4_chunk(pools4, ch,
                                                      (cp_vec, cp_act))):
                        after_sub(ch * c.TCH + (sub + 1) * P)
            es_p4.close()
            es_at.close()
            es_wo.close()

        # release static single-tile pools in LIFO order
        for f_ in reversed(free_stat):
            f_()

    nc.compile()
    return nc


def _host_inputs(c, x, wq, wk, wv, wo):
    """Shard + lay out the inputs for the cores."""
    xT = np.ascontiguousarray(x.reshape(c.T, c.DIM).T).astype(BF16)

    # even/odd split permutation within each head (q and k only)
    perm_head = np.concatenate([np.arange(0, c.HD, 2), np.arange(1, c.HD, 2)])

    def permute_heads(w):  # w: [DIM, n*HD]
        nh = w.shape[1] // c.HD
        w = w.reshape(c.DIM, nh, c.HD)[:, :, perm_head]
        return np.ascontiguousarray(w.reshape(c.DIM, nh * c.HD))

    wq_p = permute_heads(wq).astype(BF16)
    wk_p = permute_heads(wk).astype(BF16)
    wv_b = wv.astype(BF16)
    wo_b = wo.astype(BF16)

    # rope tables, even/odd-split feature-major layout: [128, S]
    hh = c.HD // 2
    inv = 1.0 / (c.THETA ** (np.arange(0, c.HD, 2, dtype=np.float64) / c.HD))
    pos = np.arange(c.S).astype(np.float64)
    ang = inv[:, None] * pos[None, :]              # [64, S]
    cosv = np.cos(ang).astype(np.float32)
    sinv = np.sin(ang).astype(np.float32)
    cosi = np.concatenate([cosv, cosv], 0)
    sini = np.concatenate([-sinv, sinv], 0)
    assert hh * 2 == P

    trilm = np.ascontiguousarray(
        np.tril(np.ones((P, P), np.float32)).T
    ).astype(BF16)                                  # [k, q]: 1 iff k<=q
    identm = np.eye(P, dtype=np.float32)

    KHC = c.KVH // c.NCORES  # kv heads per core (=1)
    in_maps = []
    for cc in range(c.NCORES):
        wk_sh = wk_p[:, cc * KHC * c.HD:(cc * KHC + 1) * c.HD]
        wv_sh = wv_b[:, cc * KHC * c.HD:(cc * KHC + 1) * c.HD]
        in_maps.append({
            "xT": xT,
            "wq": np.ascontiguousarray(wq_p[:, cc * c.QF:(cc + 1) * c.QF]),
            "wkv": np.ascontiguousarray(np.concatenate([wk_sh, wv_sh], 1)),
            "wo": np.ascontiguousarray(wo_b[cc * c.QF:(cc + 1) * c.QF, :]),
            "cosi": cosi,
            "sini": sini,
            "tril": trilm,
            "ident": identm,
        })
    return in_maps


def assemble(c, outs):
    """outs[cc]: concatenated per-segment ReduceScatter shards."""
    full = np.empty((c.T, c.DIM), np.float32)
    for cc in range(c.NCORES):
        o = np.asarray(outs[cc]).astype(np.float32)
        row0 = 0
        for t0, ntok in c.SEGS:
            nrow = ntok // c.NCORES
            full[t0 + cc * nrow:t0 + (cc + 1) * nrow] = o[row0:row0 + nrow]
            row0 += nrow
    return full.reshape(c.B, c.S, c.DIM)


def kernel(x, wq, wk, wv, wo):
    from concourse import bass_utils

    if "nc" not in _CACHE:
        _CACHE["cfg"] = make_cfg()
        _CACHE["nc"] = _build_graph(_CACHE["cfg"])
    nc = _CACHE["nc"]
    c = _CACHE["cfg"]

    in_maps = _host_inputs(
        c, np.asarray(x), np.asarray(wq), np.asarray(wk),
        np.asarray(wv), np.asarray(wo),
    )
    res = bass_utils.run_bass_kernel_spmd(
        nc, in_maps, core_ids=list(range(c.NCORES)), trace=_TRACE
    )
    _CACHE["last_results"] = res
    outs = [res.results[i]["out"] for i in range(c.NCORES)]
    return assemble(c, outs)
